# revision 48
# baseline (speedup 1.0000x reference)
"""BiCrossAttention Trainium2 kernel.

Shards the (B=2, H=8) problem across 8 NeuronCores as (batch, head-pair):
core c handles batch c//4 and heads {2*(c%4), 2*(c%4)+1}.  Each core
computes its two heads' QKV projections, both cross-attention branches,
and a partial output projection; the host sums the 4 per-batch partials
and adds the bias.

Schedule design (v2): the ACT engine (exp over all 2048^2 scores x 4
head-branches) needs ~1.0us per j-chunk and is saturated; the PE's
per-j-chunk attention work (scores + attn@V) is ~0.64us at full clock,
so projection / output-projection work is spread as PE filler across
ALL 128 j-chunk iterations to keep the PE dense (the PE p-state ramp
needs ~3us of continuous busy to reach 2.4GHz).  To make that spread
possible the two attention passes are SAME-BRANCH head pairs:
pass 0 = branch 0 (q2 over k1/v1, heads 0+1), pass 1 = branch 1 -- so
branch 1's k2/v2/q1 projections are legal filler during all of pass 0.

Device-side layout notes:
  - activations are passed pre-transposed/tiled: xT[p, kc, n] = x[n, kc*128+p]
  - matmuls run in bf16; scores are computed transposed (simT[j, i]) so
    exp feeds the attn@V matmul directly as a moving operand
  - the two heads' K=64 score matmuls occupy disjoint PE row groups
    (partitions 0-63 vs 64-127) and run concurrently; one Exp covers both
  - attn@V for head 0 uses stationary [v0|ones] -> acc rows 0..64
    (denominator row 64); head 1 uses [ones|v1] -> acc rows 63..127
    (denominator row 63), so both heads' normalized outputs land on
    their own partition lanes of outT and no cross-partition DMA shift
    is needed
  - attn@V lags the score/exp stream by 2 j-chunks so the exp latency
    is hidden by the next iteration's score matmuls and filler
  - the softmax denominator reciprocal is broadcast across partitions
    with a K=1 float32r matmul, deferred into the next slab's early
    iterations so the PE never waits on the DVE reciprocal chain
  - alpha gating is folded into the V weights on the host
"""

import sys
import types

import numpy as np

for _p in ("/opt/trn_rl_repo",):
    if _p not in sys.path:
        sys.path.append(_p)

# Register the axon NTFF profile hook if the image's antenv lacks it (needed
# only when tracing; harmless otherwise).
try:
    import antenv

    if "antenv.axon_hooks" not in sys.modules:
        try:
            import antenv.axon_hooks  # noqa: F401
        except ImportError:
            _hooks = types.ModuleType("antenv.axon_hooks")
            _hook_holder = [None]
            _hooks.set_axon_ntff_profile_hook = lambda h: _hook_holder.__setitem__(0, h)
            _hooks.get_axon_ntff_profile_hook = lambda: _hook_holder[0]
            sys.modules["antenv.axon_hooks"] = _hooks
            antenv.axon_hooks = _hooks
            try:
                from trn_agent_boot.trn_boot import _ntff_profile_via_ctypes

                _hooks.set_axon_ntff_profile_hook(
                    _ntff_profile_via_ctypes("/opt/axon/libaxon_pjrt.so")
                )
            except Exception:
                pass
except Exception:
    pass

import ml_dtypes
import concourse.bacc as bacc
import concourse.mybir as mybir
import concourse.tile as tile
from concourse import bass_utils
from concourse.masks import make_identity

F32 = mybir.dt.float32
F32R = mybir.dt.float32r
BF16 = mybir.dt.bfloat16

_NP = {F32: np.float32, F32R: np.float32, BF16: ml_dtypes.bfloat16}

# Full problem constants
B, N, QD, CD, H, DH = 2, 2048, 1024, 1024, 8, 64
INNER = H * DH
SCALE = DH**-0.5
N_CORES = 8
HG = 4  # head-groups (of 2 heads) per batch


class Cfg:
    def __init__(self, n=N, d=QD, dt_proj=BF16, dt_attn=BF16, dt_out=BF16):
        self.N = n          # sequence length
        self.D = d          # model dim (= QD = CD)
        self.KC = d // 128  # contraction chunks for projections
        self.ISLAB = min(512, n)   # attention i-slab
        self.NJC = n // 128  # j chunks (128 keys each)
        self.dt_proj = dt_proj
        self.dt_attn = dt_attn
        self.dt_out = dt_out


def build_nc(cfg: Cfg):
    """Builds the single-core program (SPMD across all 8 cores)."""
    nc = bacc.Bacc("TRN2", target_bir_lowering=False, debug=False)
    KC, Nn, D = cfg.KC, cfg.N, cfg.D
    ISLAB, NJC = cfg.ISLAB, cfg.NJC
    NSL = Nn // ISLAB
    DTP, DTA, DTO = cfg.dt_proj, cfg.dt_attn, cfg.dt_out
    LAG = 2  # attn@V lags the score/exp stream by this many j-chunks

    NCH_ = Nn // min(512, Nn)
    xT = nc.dram_tensor(
        "xT", [NCH_, 128, KC, min(512, Nn)], DTP, kind="ExternalInput"
    ).ap()
    cT = nc.dram_tensor(
        "cT", [NCH_, 128, KC, min(512, Nn)], DTP, kind="ExternalInput"
    ).ap()
    wd = {
        name: nc.dram_tensor(name, [128, KC, 128], DTP, kind="ExternalInput").ap()
        for name in ("wq1", "wk1", "wv1", "wq2", "wk2", "wv2")
    }
    wout_d = nc.dram_tensor("wout", [128, D], DTO, kind="ExternalInput").ap()
    y_d = nc.dram_tensor("y", [Nn, D], BF16, kind="ExternalOutput").ap()

    with tile.TileContext(nc) as tc:
        with (
            tc.tile_pool(name="const", bufs=1) as cpool,
            tc.tile_pool(name="qkv", bufs=1) as qkvpool,
            tc.tile_pool(name="vaug", bufs=1) as vaugpool,
            tc.tile_pool(name="outp", bufs=1) as outpool,
            tc.tile_pool(name="slab", bufs=8) as slabpool,
            tc.tile_pool(name="exp", bufs=6) as exppool,
            tc.tile_pool(name="tmp", bufs=6) as tmppool,
            tc.tile_pool(name="ysb", bufs=3) as ypool,
            tc.tile_pool(name="sim", bufs=2, space="PSUM") as simpool,
            tc.tile_pool(name="acc", bufs=2, space="PSUM") as accpool,
            tc.tile_pool(name="util", bufs=2, space="PSUM") as utilpool,
        ):
            # ---- input DMAs, spread across engine queues so the first
            # slab's critical path (q2 chunk0 <- wq2+cT0, k1/v1 chunk0 <-
            # wk1/wv1+xT0) transfers in parallel; later slabs by deadline
            # (vaug0 streams xT1-3 during pass0 slab0; cT1 by slab0 end)
            w_sb = {}

            def dma_w(name, eng):
                w_sb[name] = cpool.tile([128, KC, 128], DTP, tag=name, name=name)
                eng.dma_start(out=w_sb[name][:], in_=wd[name])

            xs_tiles = {}

            def dma_slab(stream, ch, eng):
                src = cT if stream == "2" else xT
                xs = slabpool.tile(
                    [128, KC, ISLAB], DTP, tag="xs", name=f"xs{stream}{ch}"
                )
                eng.dma_start(out=xs[:], in_=src[ch])
                xs_tiles[(stream, ch)] = xs

            # single ring, strict need order: the ring delivers in order so
            # the critical first-slab inputs are never delayed by later ones
            dma_w("wq2", nc.sync)
            dma_slab("2", 0, nc.sync)
            dma_w("wk1", nc.sync)
            dma_w("wv1", nc.sync)
            dma_slab("1", 0, nc.sync)
            dma_slab("1", 1, nc.sync)
            dma_w("wk2", nc.sync)
            dma_w("wv2", nc.sync)
            dma_w("wq1", nc.sync)
            dma_slab("2", 1, nc.sync)
            dma_slab("1", 2, nc.sync)
            dma_slab("1", 3, nc.sync)
            dma_slab("2", 2, nc.sync)
            dma_slab("2", 3, nc.sync)
            wout_sb = cpool.tile([128, D], DTO, tag="wout")
            nc.sync.dma_start(out=wout_sb[:], in_=wout_d)

            # ---- constants ----
            ident_f32 = cpool.tile([128, 128], F32, tag="ident_f32")
            make_identity(nc, ident_f32[:])
            ident = cpool.tile([128, 128], DTA, tag="ident")
            nc.vector.tensor_copy(out=ident[:], in_=ident_f32[:])
            ones_f32 = cpool.tile([128, 64], F32, tag="ones_f32")
            nc.vector.memset(ones_f32[:], 1.0)
            # bcmask rows 64/65 select den_h0 / den_h1 for the combined
            # K=2 reciprocal-broadcast matmul: rows 0-63 get 1/den0,
            # rows 64-127 get 1/den1
            bcm_f32 = cpool.tile([128, 128], F32, tag="bcm_f32")
            nc.vector.memset(bcm_f32[:], 0.0)
            nc.vector.memset(bcm_f32[64:66, 64:128], 1.0)
            nc.vector.memset(bcm_f32[64:65, 64:128], 0.0)
            nc.vector.memset(bcm_f32[64:65, 0:64], 1.0)
            bcmask = cpool.tile([128, 128], F32R, tag="bcmask")
            nc.vector.tensor_copy(out=bcmask[:], in_=bcm_f32[:])

            proj = {}
            for name in ("q1", "k1", "v1", "q2", "k2", "v2"):
                proj[name] = qkvpool.tile([128, Nn], DTA, tag=name, name=name)

            # vaug[br]: [128 j, jc, 131] = [v_h0 | ones | v_h1 | pad | ones]
            # head0 stationary = cols 0:65  -> acc rows 0-64 (den at 64)
            # head1 stationary = cols 65:131 -> acc rows 0-65 (den at 65)
            vaug = {}
            for br in (0, 1):
                vaug[br] = vaugpool.tile(
                    [128, NJC, 131], DTA, tag=f"vaug{br}", name=f"vaug{br}"
                )
                nc.vector.tensor_copy(
                    out=vaug[br][:, :, 64],
                    in_=ones_f32[:, 0:1].to_broadcast((128, NJC)),
                )
                nc.vector.memset(vaug[br][:, :, 129], 0.0)
                nc.vector.tensor_copy(
                    out=vaug[br][:, :, 130],
                    in_=ones_f32[:, 0:1].to_broadcast((128, NJC)),
                )

            outT = outpool.tile([128, Nn], DTO, tag="outT")
            outTB = outpool.tile([64, Nn], DTO, tag="outTB")

            # ---- filler job helpers (each job ~1-2K PE cycles) ----
            STREAM = {"q1": "1", "k1": "1", "v1": "1", "q2": "2", "k2": "2", "v2": "2"}

            def proj_cols(pname, c0, c1):
                """Project cols [c0:c1) of tensor pname (within one slab)."""
                xs = xs_tiles[(STREAM[pname], c0 // ISLAB)]
                off = c0 % ISLAB
                w = c1 - c0
                pp = utilpool.tile([128, 512], F32, tag="util", name=f"pp{pname}{c0}")
                wt = w_sb["w" + pname]
                for kc in range(KC):
                    nc.tensor.matmul(
                        pp[:, 0:w], wt[:, kc, :], xs[:, kc, off : off + w],
                        start=(kc == 0), stop=(kc == KC - 1),
                    )
                nc.vector.tensor_copy(out=proj[pname][:, c0:c1], in_=pp[:, 0:w])

            def vaug_cols(br, c0, c1):
                """Project v cols [c0:c1) and transpose into vaug[br]."""
                vname = "v1" if br == 0 else "v2"
                proj_cols(vname, c0, c1)
                vt = proj[vname]
                for jc in range(c0 // 128, c1 // 128):
                    pt = utilpool.tile([128, 512], DTA, tag="util", name=f"pt{br}{jc}")
                    nc.tensor.transpose(
                        pt[:, 0:128], vt[:, jc * 128 : (jc + 1) * 128], ident[:]
                    )
                    nc.vector.tensor_copy(out=vaug[br][:, jc, 0:64], in_=pt[:, 0:64])
                    nc.vector.tensor_copy(
                        out=vaug[br][:, jc, 65:129], in_=pt[:, 64:128]
                    )

            def outproj_ic(ic, scalar_copy=False):
                """Output projection for n-block ic: y[ic*128:+128, :].
                One contiguous full-row DMA per block (2KB row segments).
                scalar_copy: use the (idle-at-tail) ACT engine for the
                PSUM pulls so the DVE isn't the tail serializer."""
                icsl = slice(ic * 128, (ic + 1) * 128)
                ysb = ypool.tile([128, D], BF16, tag="ysb", name=f"ysb{ic}")
                ocw = 512
                for oc in range(D // ocw):
                    ocsl = slice(oc * ocw, (oc + 1) * ocw)
                    py = utilpool.tile([128, 512], F32, tag="util", name=f"py{ic}{oc}")
                    nc.tensor.matmul(
                        py[:, 0:ocw], outT[:, icsl], wout_sb[:, ocsl],
                        start=True, stop=True,
                    )
                    if scalar_copy:
                        nc.scalar.copy(out=ysb[:, ocsl], in_=py[:, 0:ocw])
                    else:
                        nc.vector.tensor_copy(out=ysb[:, ocsl], in_=py[:, 0:ocw])
                nc.sync.dma_start(out=y_d[icsl, :], in_=ysb[:])

            # ---- PE p-state warmup: the clock needs ~3us of continuous
            # execution to ramp to 2.4GHz; burn that in during the initial
            # DMA wait on constants so the first projections run ramped
            # sized to run right up to the first input DMA's arrival so the
            # PE enters the first projection already ramped
            warm = utilpool.tile([128, 512], F32, tag="util", name="warm")
            for w in range(72):
                nc.tensor.matmul(
                    warm[:, 0:128], ident[:], ident[:],
                    start=(w == 0), stop=(w == 71),
                )

            # ---- pre-loop: minimum work for (pass0, slab0) jc0 scores;
            # vaug0 chunk0 (first needed by attn@V at jc2) moves into the
            # joblist so the exp stream starts as soon as q2/k1 land
            proj_cols("q2", 0, ISLAB)
            proj_cols("k1", 0, 256)

            # ---- filler joblists per (pass, slab) ----
            def J(fn, *a):
                return lambda: fn(*a)

            JOBS = {}
            # pass0 slab0: stream k1/vaug0 halves just-ahead (deadline col
            # 256h needed at jc 2h), then q2 chunk1 for slab1
            lst = [J(vaug_cols, 0, 0, 256)]
            for h in range(1, 8):
                lst.append(J(proj_cols, "k1", 256 * h, 256 * h + 256))
                lst.append(J(vaug_cols, 0, 256 * h, 256 * h + 256))
            lst.append(J(proj_cols, "q2", 512, 768))
            lst.append(J(proj_cols, "q2", 768, 1024))
            JOBS[(0, 0)] = lst
            JOBS[(0, 1)] = [
                J(proj_cols, "q2", 1024, 1280), J(proj_cols, "q2", 1280, 1536),
                J(proj_cols, "k2", 0, 256), J(proj_cols, "k2", 256, 512),
                J(vaug_cols, 1, 0, 256), J(vaug_cols, 1, 256, 512),
            ]
            JOBS[(0, 2)] = [
                J(proj_cols, "q2", 1536, 1792), J(proj_cols, "q2", 1792, 2048),
                J(proj_cols, "k2", 512, 768), J(proj_cols, "k2", 768, 1024),
                J(vaug_cols, 1, 512, 768), J(vaug_cols, 1, 768, 1024),
            ]
            JOBS[(0, 3)] = [
                J(proj_cols, "q1", 0, 256), J(proj_cols, "q1", 256, 512),
                J(proj_cols, "k2", 1024, 1280), J(proj_cols, "k2", 1280, 1536),
                J(vaug_cols, 1, 1024, 1280), J(vaug_cols, 1, 1280, 1536),
            ]
            JOBS[(1, 0)] = [
                J(proj_cols, "k2", 1536, 1792), J(vaug_cols, 1, 1536, 1792),
                J(proj_cols, "k2", 1792, 2048), J(vaug_cols, 1, 1792, 2048),
                J(proj_cols, "q1", 512, 768), J(proj_cols, "q1", 768, 1024),
            ]
            JOBS[(1, 1)] = [
                J(proj_cols, "q1", 1024, 1280), J(proj_cols, "q1", 1280, 1536),
                J(outproj_ic, 0), J(outproj_ic, 1),
                J(outproj_ic, 2), J(outproj_ic, 3),
            ]
            JOBS[(1, 2)] = [
                J(proj_cols, "q1", 1536, 1792), J(proj_cols, "q1", 1792, 2048),
                J(outproj_ic, 4), J(outproj_ic, 5),
                J(outproj_ic, 6), J(outproj_ic, 7),
            ]
            JOBS[(1, 3)] = [
                J(outproj_ic, 8), J(outproj_ic, 9),
                J(outproj_ic, 10), J(outproj_ic, 11),
            ]

            normjobs = []  # deferred PE/DVE normalize tail of previous slab

            def make_norm(p, sl, acc0, acc1):
                """Emit the DVE pulls + h1 partition-shift DMA now; queue
                the PE broadcast + multiplies (+ adds for pass1) as
                normjobs.  h0 den is acc0 row 64, h1 den is acc1 row 65;
                h1's raw sums are DMA-shifted to partitions 64-127 before
                the in-place normalize multiply."""
                isl_ = slice(sl * ISLAB, (sl + 1) * ISLAB)
                if p == 0:
                    dst0 = outT[0:64, isl_]
                    stage1, dst1 = outTB[0:64, isl_], outT[64:128, isl_]
                else:
                    tmp = tmppool.tile([128, ISLAB], F32, tag="ptmp")
                    tmpB = tmppool.tile([128, ISLAB], F32, tag="ptmpB")
                    dst0 = tmp[0:64, :]
                    stage1, dst1 = tmpB[0:64, :], tmpB[64:128, :]
                # pull raw sums out of PSUM (reciprocal must run on base-0)
                nc.vector.tensor_copy(out=dst0, in_=acc0[0:64, :])
                nc.vector.tensor_copy(out=stage1, in_=acc1[0:64, :])
                rcpf0 = tmppool.tile([128, ISLAB], F32, tag="rcpf0")
                rcpf1 = tmppool.tile([128, ISLAB], F32, tag="rcpf1")
                nc.vector.reciprocal_approx_fast(out=rcpf0[0:65, :], in_=acc0[0:65, :])
                nc.vector.reciprocal_approx_fast(out=rcpf1[0:66, :], in_=acc1[0:66, :])
                rcp = tmppool.tile([128, ISLAB], F32R, tag="rcpr")
                # base-65 APs are illegal; write rows 64-65 from rcpf1 then
                # overwrite row 64 with h0's reciprocal
                nc.vector.tensor_copy(out=rcp[64:66, :], in_=rcpf1[64:66, :])
                nc.vector.tensor_copy(out=rcp[64:65, :], in_=rcpf0[64:65, :])
                # shift h1 raw sums to partitions 64-127
                nc.sync.dma_start(out=dst1, in_=stage1)

                bc_ref = []

                def bc():
                    bct = utilpool.tile(
                        [128, ISLAB], F32, tag="util", name=f"bc{p}{sl}"
                    )
                    bc_ref.append(bct)
                    nc.tensor.matmul(
                        bct[0:128, :], bcmask[64:66, :], rcp[64:66, :],
                        start=True, stop=True,
                    )

                def fin0():
                    bct = bc_ref[0]
                    nc.vector.tensor_mul(out=dst0, in0=dst0, in1=bct[0:64, :])
                    if p == 1:
                        nc.vector.tensor_add(
                            out=outT[0:64, isl_], in0=outT[0:64, isl_], in1=dst0
                        )

                def fin1():
                    bct = bc_ref[0]
                    nc.vector.tensor_mul(out=dst1, in0=dst1, in1=bct[64:128, :])
                    if p == 1:
                        nc.vector.tensor_add(
                            out=outT[64:128, isl_], in0=outT[64:128, isl_], in1=dst1
                        )

                normjobs.extend([bc, fin0, fin1])

            def tail_norm(acc0, acc1):
                """Last slab (pass1, slab NSL-1): pipelined per-n-block
                normalize + output projection to shrink the kernel tail."""
                sl = NSL - 1
                isl_ = slice(sl * ISLAB, (sl + 1) * ISLAB)
                # keep the PE ramped through the DVE/DMA waits of the tail so
                # the trailing output projections run at full clock; the warm
                # tile comes from the sim pool (its last exps are done, and
                # unlike acc/util its banks have no tail readers)
                warm2 = simpool.tile([128, 2, ISLAB], F32, tag="sim", name="warm2")

                def warm_burst(n):
                    for _ in range(n):
                        nc.tensor.matmul(
                            warm2[:, 0, 0:128], ident[:], ident[:],
                            start=True, stop=True,
                        )

                warm_burst(10)
                tmp = tmppool.tile([128, ISLAB], F32, tag="ptmp")
                tmpB = tmppool.tile([128, ISLAB], F32, tag="ptmpB")
                dst0, stage1, dst1 = tmp[0:64, :], tmpB[0:64, :], tmpB[64:128, :]
                nc.vector.tensor_copy(out=stage1, in_=acc1[0:64, :])
                nc.sync.dma_start(out=dst1, in_=stage1)
                nc.vector.tensor_copy(out=dst0, in_=acc0[0:64, :])
                rcpf0 = tmppool.tile([128, ISLAB], F32, tag="rcpf0")
                rcpf1 = tmppool.tile([128, ISLAB], F32, tag="rcpf1")
                nc.vector.reciprocal_approx_fast(out=rcpf0[0:65, :], in_=acc0[0:65, :])
                nc.vector.reciprocal_approx_fast(out=rcpf1[0:66, :], in_=acc1[0:66, :])
                rcp = tmppool.tile([128, ISLAB], F32R, tag="rcpr")
                nc.vector.tensor_copy(out=rcp[64:66, :], in_=rcpf1[64:66, :])
                nc.vector.tensor_copy(out=rcp[64:65, :], in_=rcpf0[64:65, :])
                warm_burst(8)
                bct = utilpool.tile([128, ISLAB], F32, tag="util", name="bctail")
                nc.tensor.matmul(
                    bct[0:128, :], bcmask[64:66, :], rcp[64:66, :],
                    start=True, stop=True,
                )
                base = sl * (ISLAB // 128)
                for k in range(ISLAB // 128):
                    warm_burst(3)
                    ic = base + k
                    lsl = slice(k * 128, (k + 1) * 128)
                    asl = slice(ic * 128, (ic + 1) * 128)
                    nc.vector.tensor_mul(
                        out=tmp[0:64, lsl], in0=tmp[0:64, lsl], in1=bct[0:64, lsl]
                    )
                    nc.vector.tensor_add(
                        out=outT[0:64, asl], in0=outT[0:64, asl], in1=tmp[0:64, lsl]
                    )
                    nc.vector.tensor_mul(
                        out=tmpB[64:128, lsl], in0=tmpB[64:128, lsl],
                        in1=bct[64:128, lsl],
                    )
                    nc.vector.tensor_add(
                        out=outT[64:128, asl], in0=outT[64:128, asl],
                        in1=tmpB[64:128, lsl],
                    )
                    outproj_ic(ic, scalar_copy=True)

            # ---- main attention loop, software-pipelined across slabs:
            # attn@V lags the score/exp stream by LAG j-chunks and the lag
            # carries over slab boundaries so the PE never drains (keeps
            # the p-state ramp); each slab's normalize is emitted right
            # after its last attn@V pops, early in the next slab.
            PASSES = (("q2", "k1", 0), ("q1", "k2", 1))
            pending = []

            def drain_one():
                fn, fin = pending.pop(0)
                fn()
                if fin is not None:
                    fin()

            for p, (qn, kn, br) in enumerate(PASSES):
                for sl in range(NSL):
                    isl = slice(sl * ISLAB, (sl + 1) * ISLAB)
                    acc0 = accpool.tile(
                        [128, ISLAB], F32, tag="acc", name=f"acc{p}{sl}h0"
                    )
                    acc1 = accpool.tile(
                        [128, ISLAB], F32, tag="acc", name=f"acc{p}{sl}h1"
                    )
                    joblist = list(JOBS[(p, sl)])
                    njobs = len(joblist)
                    jobs_done = 0
                    strict = p == 0 and sl == 0  # 1/jc keeps k1/vaug0 deadlines

                    def emit_attnv(jc, expP, acc0=acc0, acc1=acc1, br=br):
                        nc.tensor.matmul(
                            acc0[0:65, :], vaug[br][:, jc, 0:65], expP[:, 0, :],
                            start=(jc == 0), stop=(jc == NJC - 1),
                        )
                        nc.tensor.matmul(
                            acc1[0:66, :], vaug[br][:, jc, 65:131], expP[:, 1, :],
                            start=(jc == 0), stop=(jc == NJC - 1),
                        )

                    for jc in range(NJC):
                        jsl = slice(jc * 128, (jc + 1) * 128)
                        simP = simpool.tile([128, 2, ISLAB], F32, tag="sim")
                        for h in (0, 1):
                            rs = slice(h * 64, h * 64 + 64)
                            nc.tensor.matmul(
                                simP[:, h, :], proj[kn][rs, jsl], proj[qn][rs, isl],
                                start=True, stop=True, tile_position=(h * 64, 0),
                            )
                        expP = exppool.tile([128, 2, ISLAB], DTA, tag="exp")
                        nc.scalar.activation(
                            expP[:], simP[:],
                            mybir.ActivationFunctionType.Exp, scale=SCALE,
                        )
                        fin = None
                        if jc == NJC - 1:
                            if p == 1 and sl == NSL - 1:
                                fin = lambda a0=acc0, a1=acc1: tail_norm(a0, a1)
                            else:
                                fin = (
                                    lambda p=p, sl=sl, a0=acc0, a1=acc1:
                                    make_norm(p, sl, a0, a1)
                                )
                        pending.append(
                            (lambda jc=jc, e=expP, f=emit_attnv: f(jc, e), fin)
                        )
                        # at a slab boundary drain the previous slab's last
                        # two attn@V right away so its normalize (and the acc
                        # bank release) starts at jc0, not jc1
                        limit = LAG - 1 if jc == 0 else LAG
                        while len(pending) > limit:
                            drain_one()

                        # filler AFTER the scores: the score semaphore fires
                        # early in the period, so the exp stream never pays
                        # the cross-engine sem delay per j-chunk
                        if normjobs and jc >= 3:
                            normjobs.pop(0)()
                        if strict:
                            target = min(jc + 2, njobs)
                        else:
                            target = njobs * (jc + 1) // (NJC - 1)
                        # filler waits for the normalize tail (outproj jobs
                        # read outT, which the fin jobs finish writing)
                        while jobs_done < target and not normjobs:
                            joblist.pop(0)()
                            jobs_done += 1
                    # any leftover paced jobs act as boundary filler
                    for job in joblist:
                        job()
            while pending:
                drain_one()
            while normjobs:
                normjobs.pop(0)()

    nc.compile()
    return nc


_CACHE = {}
_ACTIVE_CFG = Cfg()


def _get_nc():
    if "nc" not in _CACHE:
        _CACHE["nc"] = build_nc(_ACTIVE_CFG)
    return _CACHE["nc"]


def _tile_kpart(a, dt):
    """[K, M] -> [128, K//128, M] with element (p, kc, m) = a[kc*128+p, m]."""
    k, m = a.shape
    return np.ascontiguousarray(
        a.reshape(k // 128, 128, m).transpose(1, 0, 2)
    ).astype(_NP[dt])


def make_in_maps(x, context, Wq1, Wk1, Wv1, Wq2, Wk2, Wv2, alpha_attn, Wout, bout):
    cfg = _ACTIVE_CFG
    alpha = float(1.0 / (1.0 + np.exp(-np.float64(alpha_attn))))
    Wv1s = np.asarray(Wv1, np.float32) * np.float32(alpha)
    Wv2s = np.asarray(Wv2, np.float32) * np.float32(1.0 - alpha)

    def _chunked(a):
        t = _tile_kpart(a, cfg.dt_proj)  # [128, KC, N]
        w = min(512, cfg.N)
        return np.ascontiguousarray(
            t.reshape(128, cfg.KC, cfg.N // w, w).transpose(2, 0, 1, 3)
        )

    xT = [_chunked(np.asarray(x[b], np.float32).T) for b in range(B)]
    cT = [_chunked(np.asarray(context[b], np.float32).T) for b in range(B)]

    in_maps = []
    for c in range(N_CORES):
        b, hg = c // HG, c % HG
        cols = slice(hg * 128, (hg + 1) * 128)
        in_maps.append(
            {
                "xT": xT[b],
                "cT": cT[b],
                "wq1": _tile_kpart(np.asarray(Wq1, np.float32)[:, cols], cfg.dt_proj),
                "wk1": _tile_kpart(np.asarray(Wk1, np.float32)[:, cols], cfg.dt_proj),
                "wv1": _tile_kpart(Wv1s[:, cols], cfg.dt_proj),
                "wq2": _tile_kpart(np.asarray(Wq2, np.float32)[:, cols], cfg.dt_proj),
                "wk2": _tile_kpart(np.asarray(Wk2, np.float32)[:, cols], cfg.dt_proj),
                "wv2": _tile_kpart(Wv2s[:, cols], cfg.dt_proj),
                "wout": np.ascontiguousarray(
                    np.asarray(Wout, np.float32)[cols, :]
                ).astype(_NP[cfg.dt_out]),
            }
        )
    return in_maps


def run_device(in_maps, trace=False, tmpdir=None):
    nc = _get_nc()
    return bass_utils.run_bass_kernel_spmd(
        nc, in_maps, core_ids=list(range(N_CORES)), trace=trace, tmpdir=tmpdir
    )


def kernel(x, context, Wq1, Wk1, Wv1, Wq2, Wk2, Wv2, alpha_attn, Wout, bout):
    in_maps = make_in_maps(
        x, context, Wq1, Wk1, Wv1, Wq2, Wk2, Wv2, alpha_attn, Wout, bout
    )
    res = run_device(in_maps)
    bout32 = np.asarray(bout, np.float32)
    out = np.empty((B, N, QD), np.float32)
    for b in range(B):
        acc = res.results[b * HG]["y"].astype(np.float32).copy()
        for hg in range(1, HG):
            acc += res.results[b * HG + hg]["y"]
        out[b] = acc + bout32[None, :]
    return out


# revision 50
# speedup vs baseline: 1.0225x; 1.0225x over previous
"""BiCrossAttention Trainium2 kernel.

Shards the (B=2, H=8) problem across 8 NeuronCores as (batch, head-pair):
core c handles batch c//4 and heads {2*(c%4), 2*(c%4)+1}.  Each core
computes its two heads' QKV projections, both cross-attention branches,
and a partial output projection; the host sums the 4 per-batch partials
and adds the bias.

Schedule design (v2): the ACT engine (exp over all 2048^2 scores x 4
head-branches) needs ~1.0us per j-chunk and is saturated; the PE's
per-j-chunk attention work (scores + attn@V) is ~0.64us at full clock,
so projection / output-projection work is spread as PE filler across
ALL 128 j-chunk iterations to keep the PE dense (the PE p-state ramp
needs ~3us of continuous busy to reach 2.4GHz).  To make that spread
possible the two attention passes are SAME-BRANCH head pairs:
pass 0 = branch 0 (q2 over k1/v1, heads 0+1), pass 1 = branch 1 -- so
branch 1's k2/v2/q1 projections are legal filler during all of pass 0.

Device-side layout notes:
  - activations are passed pre-transposed/tiled: xT[p, kc, n] = x[n, kc*128+p]
  - matmuls run in bf16; scores are computed transposed (simT[j, i]) so
    exp feeds the attn@V matmul directly as a moving operand
  - the two heads' K=64 score matmuls occupy disjoint PE row groups
    (partitions 0-63 vs 64-127) and run concurrently; one Exp covers both
  - attn@V for head 0 uses stationary [v0|ones] -> acc rows 0..64
    (denominator row 64); head 1 uses [ones|v1] -> acc rows 63..127
    (denominator row 63), so both heads' normalized outputs land on
    their own partition lanes of outT and no cross-partition DMA shift
    is needed
  - attn@V lags the score/exp stream by 2 j-chunks so the exp latency
    is hidden by the next iteration's score matmuls and filler
  - the softmax denominator reciprocal is broadcast across partitions
    with a K=1 float32r matmul, deferred into the next slab's early
    iterations so the PE never waits on the DVE reciprocal chain
  - alpha gating is folded into the V weights on the host
"""

import sys
import types

import numpy as np

for _p in ("/opt/trn_rl_repo",):
    if _p not in sys.path:
        sys.path.append(_p)

# Register the axon NTFF profile hook if the image's antenv lacks it (needed
# only when tracing; harmless otherwise).
try:
    import antenv

    if "antenv.axon_hooks" not in sys.modules:
        try:
            import antenv.axon_hooks  # noqa: F401
        except ImportError:
            _hooks = types.ModuleType("antenv.axon_hooks")
            _hook_holder = [None]
            _hooks.set_axon_ntff_profile_hook = lambda h: _hook_holder.__setitem__(0, h)
            _hooks.get_axon_ntff_profile_hook = lambda: _hook_holder[0]
            sys.modules["antenv.axon_hooks"] = _hooks
            antenv.axon_hooks = _hooks
            try:
                from trn_agent_boot.trn_boot import _ntff_profile_via_ctypes

                _hooks.set_axon_ntff_profile_hook(
                    _ntff_profile_via_ctypes("/opt/axon/libaxon_pjrt.so")
                )
            except Exception:
                pass
except Exception:
    pass

import ml_dtypes
import concourse.bacc as bacc
import concourse.mybir as mybir
import concourse.tile as tile
from concourse import bass_utils
from concourse.masks import make_identity

F32 = mybir.dt.float32
F32R = mybir.dt.float32r
BF16 = mybir.dt.bfloat16

_NP = {F32: np.float32, F32R: np.float32, BF16: ml_dtypes.bfloat16}

# Full problem constants
B, N, QD, CD, H, DH = 2, 2048, 1024, 1024, 8, 64
INNER = H * DH
SCALE = DH**-0.5
N_CORES = 8
HG = 4  # head-groups (of 2 heads) per batch


class Cfg:
    def __init__(self, n=N, d=QD, dt_proj=BF16, dt_attn=BF16, dt_out=BF16):
        self.N = n          # sequence length
        self.D = d          # model dim (= QD = CD)
        self.KC = d // 128  # contraction chunks for projections
        self.ISLAB = min(512, n)   # attention i-slab
        self.NJC = n // 128  # j chunks (128 keys each)
        self.dt_proj = dt_proj
        self.dt_attn = dt_attn
        self.dt_out = dt_out


def build_nc(cfg: Cfg):
    """Builds the single-core program (SPMD across all 8 cores)."""
    nc = bacc.Bacc("TRN2", target_bir_lowering=False, debug=False)
    KC, Nn, D = cfg.KC, cfg.N, cfg.D
    ISLAB, NJC = cfg.ISLAB, cfg.NJC
    NSL = Nn // ISLAB
    DTP, DTA, DTO = cfg.dt_proj, cfg.dt_attn, cfg.dt_out
    LAG = 2  # attn@V lags the score/exp stream by this many j-chunks

    NCH_ = Nn // min(512, Nn)
    xT = nc.dram_tensor(
        "xT", [NCH_, 128, KC, min(512, Nn)], DTP, kind="ExternalInput"
    ).ap()
    cT = nc.dram_tensor(
        "cT", [NCH_, 128, KC, min(512, Nn)], DTP, kind="ExternalInput"
    ).ap()
    wd = {
        name: nc.dram_tensor(name, [128, KC, 128], DTP, kind="ExternalInput").ap()
        for name in ("wq1", "wk1", "wv1", "wq2", "wk2", "wv2")
    }
    wout_d = nc.dram_tensor("wout", [128, D], DTO, kind="ExternalInput").ap()
    y_d = nc.dram_tensor("y", [Nn, D], BF16, kind="ExternalOutput").ap()

    with tile.TileContext(nc) as tc:
        with (
            tc.tile_pool(name="const", bufs=1) as cpool,
            tc.tile_pool(name="qkv", bufs=1) as qkvpool,
            tc.tile_pool(name="vaug", bufs=1) as vaugpool,
            tc.tile_pool(name="outp", bufs=1) as outpool,
            tc.tile_pool(name="slab", bufs=8) as slabpool,
            tc.tile_pool(name="exp", bufs=6) as exppool,
            tc.tile_pool(name="tmp", bufs=6) as tmppool,
            tc.tile_pool(name="ysb", bufs=3) as ypool,
            tc.tile_pool(name="sim", bufs=2, space="PSUM") as simpool,
            tc.tile_pool(name="acc", bufs=2, space="PSUM") as accpool,
            tc.tile_pool(name="util", bufs=2, space="PSUM") as utilpool,
        ):
            # ---- input DMAs, spread across engine queues so the first
            # slab's critical path (q2 chunk0 <- wq2+cT0, k1/v1 chunk0 <-
            # wk1/wv1+xT0) transfers in parallel; later slabs by deadline
            # (vaug0 streams xT1-3 during pass0 slab0; cT1 by slab0 end)
            w_sb = {}

            def dma_w(name, eng):
                w_sb[name] = cpool.tile([128, KC, 128], DTP, tag=name, name=name)
                eng.dma_start(out=w_sb[name][:], in_=wd[name])

            xs_tiles = {}

            def dma_slab(stream, ch, eng):
                src = cT if stream == "2" else xT
                xs = slabpool.tile(
                    [128, KC, ISLAB], DTP, tag="xs", name=f"xs{stream}{ch}"
                )
                eng.dma_start(out=xs[:], in_=src[ch])
                xs_tiles[(stream, ch)] = xs

            # single ring, strict need order: the ring delivers in order so
            # the critical first-slab inputs are never delayed by later ones
            dma_w("wq2", nc.sync)
            dma_slab("2", 0, nc.sync)
            dma_w("wk1", nc.sync)
            dma_w("wv1", nc.sync)
            dma_slab("1", 0, nc.sync)
            dma_slab("1", 1, nc.sync)
            dma_w("wk2", nc.sync)
            dma_w("wv2", nc.sync)
            dma_w("wq1", nc.sync)
            dma_slab("2", 1, nc.sync)
            dma_slab("1", 2, nc.sync)
            dma_slab("1", 3, nc.sync)
            dma_slab("2", 2, nc.sync)
            dma_slab("2", 3, nc.sync)
            wout_sb = cpool.tile([128, D], DTO, tag="wout")
            nc.sync.dma_start(out=wout_sb[:], in_=wout_d)

            # ---- constants ----
            ident_f32 = cpool.tile([128, 128], F32, tag="ident_f32")
            make_identity(nc, ident_f32[:])
            ident = cpool.tile([128, 128], DTA, tag="ident")
            nc.vector.tensor_copy(out=ident[:], in_=ident_f32[:])
            ones_f32 = cpool.tile([128, 64], F32, tag="ones_f32")
            nc.vector.memset(ones_f32[:], 1.0)
            # bcmask rows 64/65 select den_h0 / den_h1 for the combined
            # K=2 reciprocal-broadcast matmul: rows 0-63 get 1/den0,
            # rows 64-127 get 1/den1
            bcm_f32 = cpool.tile([128, 128], F32, tag="bcm_f32")
            nc.vector.memset(bcm_f32[:], 0.0)
            nc.vector.memset(bcm_f32[64:66, 64:128], 1.0)
            nc.vector.memset(bcm_f32[64:65, 64:128], 0.0)
            nc.vector.memset(bcm_f32[64:65, 0:64], 1.0)
            bcmask = cpool.tile([128, 128], F32R, tag="bcmask")
            nc.vector.tensor_copy(out=bcmask[:], in_=bcm_f32[:])

            proj = {}
            for name in ("q1", "k1", "v1", "q2", "k2", "v2"):
                proj[name] = qkvpool.tile([128, Nn], DTA, tag=name, name=name)

            # vaug[br]: [128 j, jc, 131] = [v_h0 | ones | v_h1 | pad | ones]
            # head0 stationary = cols 0:65  -> acc rows 0-64 (den at 64)
            # head1 stationary = cols 65:131 -> acc rows 0-65 (den at 65)
            vaug = {}
            for br in (0, 1):
                vaug[br] = vaugpool.tile(
                    [128, NJC, 131], DTA, tag=f"vaug{br}", name=f"vaug{br}"
                )
                nc.vector.tensor_copy(
                    out=vaug[br][:, :, 64],
                    in_=ones_f32[:, 0:1].to_broadcast((128, NJC)),
                )
                nc.vector.memset(vaug[br][:, :, 129], 0.0)
                nc.vector.tensor_copy(
                    out=vaug[br][:, :, 130],
                    in_=ones_f32[:, 0:1].to_broadcast((128, NJC)),
                )

            outT = outpool.tile([128, Nn], DTO, tag="outT")
            outTB = outpool.tile([64, Nn], DTO, tag="outTB")

            # ---- filler job helpers (each job ~1-2K PE cycles) ----
            STREAM = {"q1": "1", "k1": "1", "v1": "1", "q2": "2", "k2": "2", "v2": "2"}

            def proj_cols(pname, c0, c1):
                """Project cols [c0:c1) of tensor pname (within one slab)."""
                xs = xs_tiles[(STREAM[pname], c0 // ISLAB)]
                off = c0 % ISLAB
                w = c1 - c0
                pp = utilpool.tile([128, 512], F32, tag="util", name=f"pp{pname}{c0}")
                wt = w_sb["w" + pname]
                for kc in range(KC):
                    nc.tensor.matmul(
                        pp[:, 0:w], wt[:, kc, :], xs[:, kc, off : off + w],
                        start=(kc == 0), stop=(kc == KC - 1),
                    )
                nc.vector.tensor_copy(out=proj[pname][:, c0:c1], in_=pp[:, 0:w])

            def vaug_cols(br, c0, c1):
                """Project v cols [c0:c1) and transpose into vaug[br]."""
                vname = "v1" if br == 0 else "v2"
                proj_cols(vname, c0, c1)
                vt = proj[vname]
                for jc in range(c0 // 128, c1 // 128):
                    pt = utilpool.tile([128, 512], DTA, tag="util", name=f"pt{br}{jc}")
                    nc.tensor.transpose(
                        pt[:, 0:128], vt[:, jc * 128 : (jc + 1) * 128], ident[:]
                    )
                    nc.vector.tensor_copy(out=vaug[br][:, jc, 0:64], in_=pt[:, 0:64])
                    nc.vector.tensor_copy(
                        out=vaug[br][:, jc, 65:129], in_=pt[:, 64:128]
                    )

            def outproj_ic(ic, scalar_copy=False):
                """Output projection for n-block ic: y[ic*128:+128, :].
                One contiguous full-row DMA per block (2KB row segments).
                scalar_copy: use the (idle-at-tail) ACT engine for the
                PSUM pulls so the DVE isn't the tail serializer."""
                icsl = slice(ic * 128, (ic + 1) * 128)
                ysb = ypool.tile([128, D], BF16, tag="ysb", name=f"ysb{ic}")
                ocw = 512
                for oc in range(D // ocw):
                    ocsl = slice(oc * ocw, (oc + 1) * ocw)
                    py = utilpool.tile([128, 512], F32, tag="util", name=f"py{ic}{oc}")
                    nc.tensor.matmul(
                        py[:, 0:ocw], outT[:, icsl], wout_sb[:, ocsl],
                        start=True, stop=True,
                    )
                    if scalar_copy:
                        nc.scalar.copy(out=ysb[:, ocsl], in_=py[:, 0:ocw])
                    else:
                        nc.vector.tensor_copy(out=ysb[:, ocsl], in_=py[:, 0:ocw])
                # tail blocks drain on the otherwise-idle scalar ring so the
                # final y transfers overlap the sync ring's h1 lane shift
                (nc.scalar if scalar_copy else nc.sync).dma_start(
                    out=y_d[icsl, :], in_=ysb[:]
                )

            # ---- PE p-state warmup: the clock needs ~3us of continuous
            # execution to ramp to 2.4GHz; burn that in during the initial
            # DMA wait on constants so the first projections run ramped
            # sized to run right up to the first input DMA's arrival so the
            # PE enters the first projection already ramped
            warm = utilpool.tile([128, 512], F32, tag="util", name="warm")
            for w in range(72):
                nc.tensor.matmul(
                    warm[:, 0:128], ident[:], ident[:],
                    start=(w == 0), stop=(w == 71),
                )

            # ---- pre-loop: minimum work for (pass0, slab0, jc0..1) ----
            proj_cols("q2", 0, ISLAB)
            proj_cols("k1", 0, 256)
            vaug_cols(0, 0, 256)

            # ---- filler joblists per (pass, slab) ----
            def J(fn, *a):
                return lambda: fn(*a)

            JOBS = {}
            # pass0 slab0: stream k1/vaug0 halves just-ahead (deadline col
            # 256h needed at jc 2h), then q2 chunk1 for slab1
            lst = []
            for h in range(1, 8):
                lst.append(J(proj_cols, "k1", 256 * h, 256 * h + 256))
                lst.append(J(vaug_cols, 0, 256 * h, 256 * h + 256))
            lst.append(J(proj_cols, "q2", 512, 768))
            lst.append(J(proj_cols, "q2", 768, 1024))
            JOBS[(0, 0)] = lst
            JOBS[(0, 1)] = [
                J(proj_cols, "q2", 1024, 1280), J(proj_cols, "q2", 1280, 1536),
                J(proj_cols, "k2", 0, 256), J(proj_cols, "k2", 256, 512),
                J(vaug_cols, 1, 0, 256), J(vaug_cols, 1, 256, 512),
            ]
            JOBS[(0, 2)] = [
                J(proj_cols, "q2", 1536, 1792), J(proj_cols, "q2", 1792, 2048),
                J(proj_cols, "k2", 512, 768), J(proj_cols, "k2", 768, 1024),
                J(vaug_cols, 1, 512, 768), J(vaug_cols, 1, 768, 1024),
            ]
            JOBS[(0, 3)] = [
                J(proj_cols, "q1", 0, 256), J(proj_cols, "q1", 256, 512),
                J(proj_cols, "k2", 1024, 1280), J(proj_cols, "k2", 1280, 1536),
                J(vaug_cols, 1, 1024, 1280), J(vaug_cols, 1, 1280, 1536),
            ]
            JOBS[(1, 0)] = [
                J(proj_cols, "k2", 1536, 1792), J(vaug_cols, 1, 1536, 1792),
                J(proj_cols, "k2", 1792, 2048), J(vaug_cols, 1, 1792, 2048),
                J(proj_cols, "q1", 512, 768), J(proj_cols, "q1", 768, 1024),
            ]
            JOBS[(1, 1)] = [
                J(proj_cols, "q1", 1024, 1280), J(proj_cols, "q1", 1280, 1536),
                J(outproj_ic, 0), J(outproj_ic, 1),
                J(outproj_ic, 2), J(outproj_ic, 3),
            ]
            JOBS[(1, 2)] = [
                J(proj_cols, "q1", 1536, 1792), J(proj_cols, "q1", 1792, 2048),
                J(outproj_ic, 4), J(outproj_ic, 5),
                J(outproj_ic, 6), J(outproj_ic, 7),
            ]
            JOBS[(1, 3)] = [
                J(outproj_ic, 8), J(outproj_ic, 9),
                J(outproj_ic, 10), J(outproj_ic, 11),
            ]

            normjobs = []  # deferred PE/DVE normalize tail of previous slab

            def make_norm(p, sl, acc0, acc1):
                """Emit the DVE pulls + h1 partition-shift DMA now; queue
                the PE broadcast + multiplies (+ adds for pass1) as
                normjobs.  h0 den is acc0 row 64, h1 den is acc1 row 65;
                h1's raw sums are DMA-shifted to partitions 64-127 before
                the in-place normalize multiply."""
                isl_ = slice(sl * ISLAB, (sl + 1) * ISLAB)
                if p == 0:
                    dst0 = outT[0:64, isl_]
                    stage1, dst1 = outTB[0:64, isl_], outT[64:128, isl_]
                else:
                    tmp = tmppool.tile([128, ISLAB], F32, tag="ptmp")
                    tmpB = tmppool.tile([128, ISLAB], F32, tag="ptmpB")
                    dst0 = tmp[0:64, :]
                    stage1, dst1 = tmpB[0:64, :], tmpB[64:128, :]
                # pull raw sums out of PSUM (reciprocal must run on base-0)
                nc.vector.tensor_copy(out=dst0, in_=acc0[0:64, :])
                nc.vector.tensor_copy(out=stage1, in_=acc1[0:64, :])
                rcpf0 = tmppool.tile([128, ISLAB], F32, tag="rcpf0")
                rcpf1 = tmppool.tile([128, ISLAB], F32, tag="rcpf1")
                nc.vector.reciprocal_approx_fast(out=rcpf0[0:65, :], in_=acc0[0:65, :])
                nc.vector.reciprocal_approx_fast(out=rcpf1[0:66, :], in_=acc1[0:66, :])
                rcp = tmppool.tile([128, ISLAB], F32R, tag="rcpr")
                # base-65 APs are illegal; write rows 64-65 from rcpf1 then
                # overwrite row 64 with h0's reciprocal
                nc.vector.tensor_copy(out=rcp[64:66, :], in_=rcpf1[64:66, :])
                nc.vector.tensor_copy(out=rcp[64:65, :], in_=rcpf0[64:65, :])
                # shift h1 raw sums to partitions 64-127
                nc.sync.dma_start(out=dst1, in_=stage1)

                bc_ref = []

                def bc():
                    bct = utilpool.tile(
                        [128, ISLAB], F32, tag="util", name=f"bc{p}{sl}"
                    )
                    bc_ref.append(bct)
                    nc.tensor.matmul(
                        bct[0:128, :], bcmask[64:66, :], rcp[64:66, :],
                        start=True, stop=True,
                    )

                def fin0():
                    bct = bc_ref[0]
                    nc.vector.tensor_mul(out=dst0, in0=dst0, in1=bct[0:64, :])
                    if p == 1:
                        nc.vector.tensor_add(
                            out=outT[0:64, isl_], in0=outT[0:64, isl_], in1=dst0
                        )

                def fin1():
                    bct = bc_ref[0]
                    nc.vector.tensor_mul(out=dst1, in0=dst1, in1=bct[64:128, :])
                    if p == 1:
                        nc.vector.tensor_add(
                            out=outT[64:128, isl_], in0=outT[64:128, isl_], in1=dst1
                        )

                normjobs.extend([bc, fin0, fin1])

            def tail_norm(acc0, acc1):
                """Last slab (pass1, slab NSL-1): pipelined per-n-block
                normalize + output projection to shrink the kernel tail."""
                sl = NSL - 1
                isl_ = slice(sl * ISLAB, (sl + 1) * ISLAB)
                # keep the PE ramped through the DVE/DMA waits of the tail so
                # the trailing output projections run at full clock; the warm
                # tile comes from the sim pool (its last exps are done, and
                # unlike acc/util its banks have no tail readers)
                warm2 = simpool.tile([128, 2, ISLAB], F32, tag="sim", name="warm2")

                def warm_burst(n):
                    for _ in range(n):
                        nc.tensor.matmul(
                            warm2[:, 0, 0:128], ident[:], ident[:],
                            start=True, stop=True,
                        )

                warm_burst(10)
                tmp = tmppool.tile([128, ISLAB], F32, tag="ptmp")
                tmpB = tmppool.tile([128, ISLAB], F32, tag="ptmpB")
                dst0, stage1, dst1 = tmp[0:64, :], tmpB[0:64, :], tmpB[64:128, :]
                nc.vector.tensor_copy(out=stage1, in_=acc1[0:64, :])
                nc.sync.dma_start(out=dst1, in_=stage1)
                nc.vector.tensor_copy(out=dst0, in_=acc0[0:64, :])
                rcpf0 = tmppool.tile([128, ISLAB], F32, tag="rcpf0")
                rcpf1 = tmppool.tile([128, ISLAB], F32, tag="rcpf1")
                nc.vector.reciprocal_approx_fast(out=rcpf0[0:65, :], in_=acc0[0:65, :])
                nc.vector.reciprocal_approx_fast(out=rcpf1[0:66, :], in_=acc1[0:66, :])
                rcp = tmppool.tile([128, ISLAB], F32R, tag="rcpr")
                nc.vector.tensor_copy(out=rcp[64:66, :], in_=rcpf1[64:66, :])
                nc.vector.tensor_copy(out=rcp[64:65, :], in_=rcpf0[64:65, :])
                warm_burst(8)
                bct = utilpool.tile([128, ISLAB], F32, tag="util", name="bctail")
                nc.tensor.matmul(
                    bct[0:128, :], bcmask[64:66, :], rcp[64:66, :],
                    start=True, stop=True,
                )
                base = sl * (ISLAB // 128)
                for k in range(ISLAB // 128):
                    warm_burst(3)
                    ic = base + k
                    lsl = slice(k * 128, (k + 1) * 128)
                    asl = slice(ic * 128, (ic + 1) * 128)
                    nc.vector.tensor_mul(
                        out=tmp[0:64, lsl], in0=tmp[0:64, lsl], in1=bct[0:64, lsl]
                    )
                    nc.vector.tensor_add(
                        out=outT[0:64, asl], in0=outT[0:64, asl], in1=tmp[0:64, lsl]
                    )
                    nc.vector.tensor_mul(
                        out=tmpB[64:128, lsl], in0=tmpB[64:128, lsl],
                        in1=bct[64:128, lsl],
                    )
                    nc.vector.tensor_add(
                        out=outT[64:128, asl], in0=outT[64:128, asl],
                        in1=tmpB[64:128, lsl],
                    )
                    outproj_ic(ic, scalar_copy=True)

            # ---- main attention loop, software-pipelined across slabs:
            # attn@V lags the score/exp stream by LAG j-chunks and the lag
            # carries over slab boundaries so the PE never drains (keeps
            # the p-state ramp); each slab's normalize is emitted right
            # after its last attn@V pops, early in the next slab.
            PASSES = (("q2", "k1", 0), ("q1", "k2", 1))
            pending = []

            def drain_one():
                fn, fin = pending.pop(0)
                fn()
                if fin is not None:
                    fin()

            for p, (qn, kn, br) in enumerate(PASSES):
                for sl in range(NSL):
                    isl = slice(sl * ISLAB, (sl + 1) * ISLAB)
                    acc0 = accpool.tile(
                        [128, ISLAB], F32, tag="acc", name=f"acc{p}{sl}h0"
                    )
                    acc1 = accpool.tile(
                        [128, ISLAB], F32, tag="acc", name=f"acc{p}{sl}h1"
                    )
                    joblist = list(JOBS[(p, sl)])
                    njobs = len(joblist)
                    jobs_done = 0
                    strict = p == 0 and sl == 0  # 1/jc keeps k1/vaug0 deadlines

                    def emit_attnv(jc, expP, acc0=acc0, acc1=acc1, br=br):
                        nc.tensor.matmul(
                            acc0[0:65, :], vaug[br][:, jc, 0:65], expP[:, 0, :],
                            start=(jc == 0), stop=(jc == NJC - 1),
                        )
                        nc.tensor.matmul(
                            acc1[0:66, :], vaug[br][:, jc, 65:131], expP[:, 1, :],
                            start=(jc == 0), stop=(jc == NJC - 1),
                        )

                    for jc in range(NJC):
                        jsl = slice(jc * 128, (jc + 1) * 128)
                        simP = simpool.tile([128, 2, ISLAB], F32, tag="sim")
                        for h in (0, 1):
                            rs = slice(h * 64, h * 64 + 64)
                            nc.tensor.matmul(
                                simP[:, h, :], proj[kn][rs, jsl], proj[qn][rs, isl],
                                start=True, stop=True, tile_position=(h * 64, 0),
                            )
                        expP = exppool.tile([128, 2, ISLAB], DTA, tag="exp")
                        nc.scalar.activation(
                            expP[:], simP[:],
                            mybir.ActivationFunctionType.Exp, scale=SCALE,
                        )
                        fin = None
                        if jc == NJC - 1:
                            if p == 1 and sl == NSL - 1:
                                fin = lambda a0=acc0, a1=acc1: tail_norm(a0, a1)
                            else:
                                fin = (
                                    lambda p=p, sl=sl, a0=acc0, a1=acc1:
                                    make_norm(p, sl, a0, a1)
                                )
                        pending.append(
                            (lambda jc=jc, e=expP, f=emit_attnv: f(jc, e), fin)
                        )
                        # at a slab boundary drain the previous slab's last
                        # two attn@V right away so its normalize (and the acc
                        # bank release) starts at jc0, not jc1
                        limit = LAG - 1 if jc == 0 else LAG
                        while len(pending) > limit:
                            drain_one()

                        # filler AFTER the scores: the score semaphore fires
                        # early in the period, so the exp stream never pays
                        # the cross-engine sem delay per j-chunk
                        if normjobs and jc >= 3:
                            normjobs.pop(0)()
                        if strict:
                            target = min(jc + 1, njobs)
                        else:
                            target = njobs * (jc + 1) // (NJC - 1)
                        # filler waits for the normalize tail (outproj jobs
                        # read outT, which the fin jobs finish writing)
                        while jobs_done < target and not normjobs:
                            joblist.pop(0)()
                            jobs_done += 1
                    # any leftover paced jobs act as boundary filler
                    for job in joblist:
                        job()
            while pending:
                drain_one()
            while normjobs:
                normjobs.pop(0)()

    nc.compile()
    return nc


_CACHE = {}
_ACTIVE_CFG = Cfg()


def _get_nc():
    if "nc" not in _CACHE:
        _CACHE["nc"] = build_nc(_ACTIVE_CFG)
    return _CACHE["nc"]


def _tile_kpart(a, dt):
    """[K, M] -> [128, K//128, M] with element (p, kc, m) = a[kc*128+p, m]."""
    k, m = a.shape
    return np.ascontiguousarray(
        a.reshape(k // 128, 128, m).transpose(1, 0, 2)
    ).astype(_NP[dt])


def make_in_maps(x, context, Wq1, Wk1, Wv1, Wq2, Wk2, Wv2, alpha_attn, Wout, bout):
    cfg = _ACTIVE_CFG
    alpha = float(1.0 / (1.0 + np.exp(-np.float64(alpha_attn))))
    Wv1s = np.asarray(Wv1, np.float32) * np.float32(alpha)
    Wv2s = np.asarray(Wv2, np.float32) * np.float32(1.0 - alpha)

    def _chunked(a):
        t = _tile_kpart(a, cfg.dt_proj)  # [128, KC, N]
        w = min(512, cfg.N)
        return np.ascontiguousarray(
            t.reshape(128, cfg.KC, cfg.N // w, w).transpose(2, 0, 1, 3)
        )

    xT = [_chunked(np.asarray(x[b], np.float32).T) for b in range(B)]
    cT = [_chunked(np.asarray(context[b], np.float32).T) for b in range(B)]

    in_maps = []
    for c in range(N_CORES):
        b, hg = c // HG, c % HG
        cols = slice(hg * 128, (hg + 1) * 128)
        in_maps.append(
            {
                "xT": xT[b],
                "cT": cT[b],
                "wq1": _tile_kpart(np.asarray(Wq1, np.float32)[:, cols], cfg.dt_proj),
                "wk1": _tile_kpart(np.asarray(Wk1, np.float32)[:, cols], cfg.dt_proj),
                "wv1": _tile_kpart(Wv1s[:, cols], cfg.dt_proj),
                "wq2": _tile_kpart(np.asarray(Wq2, np.float32)[:, cols], cfg.dt_proj),
                "wk2": _tile_kpart(np.asarray(Wk2, np.float32)[:, cols], cfg.dt_proj),
                "wv2": _tile_kpart(Wv2s[:, cols], cfg.dt_proj),
                "wout": np.ascontiguousarray(
                    np.asarray(Wout, np.float32)[cols, :]
                ).astype(_NP[cfg.dt_out]),
            }
        )
    return in_maps


def run_device(in_maps, trace=False, tmpdir=None):
    nc = _get_nc()
    return bass_utils.run_bass_kernel_spmd(
        nc, in_maps, core_ids=list(range(N_CORES)), trace=trace, tmpdir=tmpdir
    )


def kernel(x, context, Wq1, Wk1, Wv1, Wq2, Wk2, Wv2, alpha_attn, Wout, bout):
    in_maps = make_in_maps(
        x, context, Wq1, Wk1, Wv1, Wq2, Wk2, Wv2, alpha_attn, Wout, bout
    )
    res = run_device(in_maps)
    bout32 = np.asarray(bout, np.float32)
    out = np.empty((B, N, QD), np.float32)
    for b in range(B):
        acc = res.results[b * HG]["y"].astype(np.float32).copy()
        for hg in range(1, HG):
            acc += res.results[b * HG + hg]["y"]
        out[b] = acc + bout32[None, :]
    return out
